# revision 29
# baseline (speedup 1.0000x reference)
"""Bass/Tile TRN2 kernel for nn_AttentionANEWraperChannelsFirstWithCache.

Tensor-parallel over heads across 8 NeuronCores (v3):
  - 28 q heads in 4 slots/core (odd cores carry 1 zero dummy); core c owns
    kv head c//2.
  - All inputs host-packed partition-major so every DMA is a few large
    contiguous descriptors; K cache pre-transposed to [d, s] on the host.
  - Projection stream: interleaved [x_t | wq_t | wk_t | wv_t] chunks on the
    sync DGE queue; K/V cache queued behind it; trig/wo on the scalar queue.
  - Attention in slot pairs, software-pipelined on PE (scores for st issue
    ahead of out-MMs for st-1 so exp latency never blocks the PE queue).
    Softmax denominator: one DVE pair-fold [128,1024] + one f32 accumulate
    per 2 st. GPSIMD carries nothing but the collectives.
  - Four slot-level AllGathers; o_proj (row-parallel, 448 rows/core) runs
    as 8-matmul passes interleaved into later attention and gather waits.
"""

import math
import numpy as np

H, KV, HD, LI = 28, 4, 128, 5
S_MAX, D, L = 4096, 3584, 512
NCORES = 8
SLOTS = 4
OSH = D // NCORES          # 448 o_proj output rows per core
NT = D // 128              # 28 contraction tiles
ST = S_MAX // 128          # 32 s-tiles
SCALE = 1.0 / math.sqrt(HD)

PAIRS = [(0, 1), (2, 3)]


def _head_of(core, slot):
    off = 4 * (core % 2) + slot
    if off >= 7:
        return None
    return (core // 2) * 7 + off


# o_proj / gather order: slot-major
REAL_JC = [(j, c) for j in range(SLOTS) for c in range(NCORES)
           if _head_of(c, j) is not None]

_prog_cache = {}


def _build(cp):
    import concourse.bass as bass
    import concourse.mybir as mybir
    import concourse.tile as tile
    from concourse import bacc
    from contextlib import ExitStack

    f32 = mybir.dt.float32
    bf = mybir.dt.bfloat16
    AF = mybir.ActivationFunctionType
    nc = bacc.Bacc("TRN2", target_bir_lowering=False, debug=False,
                   num_devices=NCORES)

    wt0 = cp // 128

    proj_d = nc.dram_tensor("proj", [128, NT, 1280], bf, kind="ExternalInput")
    trig_d = nc.dram_tensor("trig", [128, 4, L], bf, kind="ExternalInput")
    bias_d = nc.dram_tensor("biases", [128, 6], f32, kind="ExternalInput")
    idrot_d = nc.dram_tensor("idrot", [128, 2, HD], bf, kind="ExternalInput")
    kT_d = nc.dram_tensor("kT", [128, S_MAX], bf, kind="ExternalInput")
    v_d = nc.dram_tensor("vc", [128, ST, HD], bf, kind="ExternalInput")
    woT_d = nc.dram_tensor("woT", [128, len(REAL_JC), OSH], bf,
                           kind="ExternalInput")
    out_d = nc.dram_tensor("out", [128, 4 * L], f32, kind="ExternalOutput")

    with tile.TileContext(nc) as tc, ExitStack() as ctx:
        const = ctx.enter_context(tc.tile_pool(name="const", bufs=1))
        kvpool = ctx.enter_context(tc.tile_pool(name="kvpool", bufs=1))
        wopool = ctx.enter_context(tc.tile_pool(name="wopool", bufs=1))
        agpool = ctx.enter_context(tc.tile_pool(name="agpool", bufs=1))
        qpool = ctx.enter_context(tc.tile_pool(name="qpool", bufs=1))
        dram = ctx.enter_context(tc.tile_pool(name="dram", bufs=1, space="DRAM"))

        # gather buffers: one 2-slot gather for pair 0, per-slot gathers for
        # slots 2/3 (pipelines the mesh sends against o_proj passes)
        ag_in = {0: dram.tile([2 * HD, L], bf, tag="agin0", name="ag_in0"),
                 2: dram.tile([HD, L], bf, tag="agin2", name="ag_in2"),
                 3: dram.tile([HD, L], bf, tag="agin3", name="ag_in3")}
        ag_out = {0: dram.tile([NCORES * 2 * HD, L], bf, tag="agout0",
                               name="ag_out0", addr_space="Shared"),
                  2: dram.tile([NCORES * HD, L], bf, tag="agout2",
                               name="ag_out2", addr_space="Shared"),
                  3: dram.tile([NCORES * HD, L], bf, tag="agout3",
                               name="ag_out3", addr_space="Shared")}

        # ---- persistent SBUF ----
        K_T = kvpool.tile([128, S_MAX], bf, tag="kt", name="K_T")      # [d, s]
        v_sb = kvpool.tile([128, ST, HD], bf, tag="v", name="v_sb")    # [s,t,d]
        woT_sb = wopool.tile([128, len(REAL_JC), OSH], bf, tag="wo",
                             name="woT_sb")
        trig = const.tile([128, 4, L], bf, tag="trig", name="trig")
        bia = const.tile([128, 6], f32, tag="bia", name="bia")
        idrot = const.tile([128, 2, HD], bf, tag="idrot", name="idrot")
        ones_f = const.tile([128, 1], f32, tag="ones_f", name="ones_f")
        onesr_f = const.tile([1, 128], f32, tag="onesr_f", name="onesr_f")
        warm = const.tile([128, L], bf, tag="warm", name="warm")
        escr = const.tile([1, 8], bf, tag="escr", name="escr")
        q_sb = [qpool.tile([128, L], bf, tag=f"q{j}", name=f"q_sb{j}")
                for j in range(SLOTS)]
        o_acc = qpool.tile([128, 4, L], f32, tag="oacc", name="o_acc")

        # ---- DMA issue ----
        # scalar DGE queue: small constants only; everything bulky goes on
        # the sync queue *behind* the projection stream so the stream that
        # feeds the PE never loses HBM bandwidth
        nc.scalar.dma_start(out=trig[:], in_=trig_d[:])
        nc.scalar.dma_start(out=bia[:], in_=bias_d[:])
        nc.scalar.dma_start(out=idrot[:], in_=idrot_d[:])

        # engine warmers: PE matmul spin (HAM warm-up) + exp table preload
        nc.gpsimd.memset(warm[:], 1.0)
        nc.gpsimd.memset(ones_f[:], 1.0)
        nc.gpsimd.memset(onesr_f[:], 1.0)
        nc.scalar.activation(escr[:], warm[0:1, 0:8], AF.Exp, scale=1.0)

        qcos, qsin = trig[:, 0, :], trig[:, 1, :]
        kcos, ksin = trig[:, 2, :], trig[:, 3, :]
        ident, rotm = idrot[:, 0, :], idrot[:, 1, :]

        scopeA = ExitStack()
        with scopeA:
            projpool = scopeA.enter_context(tc.tile_pool(name="projpool", bufs=6))
            tmppool = scopeA.enter_context(tc.tile_pool(name="tmppool", bufs=4))
            pp = scopeA.enter_context(tc.tile_pool(name="pp", bufs=1, space="PSUM"))

            # PE warmup spin on the kv psum ring (allocated first so the real
            # k_ps/v_ps wait on the warmup, never the reverse)
            for i in range(2):
                wps = pp.tile([128, L], f32, tag="kv", bufs=2,
                              name=f"warm_ps{i}")
                for _ in range(8):
                    nc.tensor.matmul(wps[:], lhsT=warm[:, 0:128], rhs=warm[:],
                                     start=True, stop=True)

            q_ps = [pp.tile([128, L], f32, tag=f"pq{j}", name=f"q_ps{j}")
                    for j in range(SLOTS)]
            kv_ps = [pp.tile([128, L], f32, tag="kv", bufs=2, name=f"kv_ps{i}")
                     for i in range(2)]
            k_ps, v_ps = kv_ps

            # ---- projection stream (sync queue): 14 chunks of 2 t-tiles,
            #      deep ring; caches + wo weights queued behind the stream
            #      so they never steal bandwidth from it ----
            CH = 2
            for ck in range(NT // CH):
                pj = projpool.tile([128, CH, 1280], bf, tag="pj",
                                   name=f"pj{ck}")
                nc.sync.dma_start(out=pj[:], in_=proj_d[:, ck * CH:(ck + 1) * CH])
                for i in range(CH):
                    t = ck * CH + i
                    first, last = t == 0, t == NT - 1
                    xv = pj[:, i, 0:512]
                    for j in range(SLOTS):
                        nc.tensor.matmul(
                            q_ps[j][:],
                            lhsT=pj[:, i, 512 + j * 128:640 + j * 128],
                            rhs=xv, start=first, stop=last)
                    nc.tensor.matmul(k_ps[:], lhsT=pj[:, i, 1024:1152],
                                     rhs=xv, start=first, stop=last)
                    nc.tensor.matmul(v_ps[:], lhsT=pj[:, i, 1152:1280],
                                     rhs=xv, start=first, stop=last)
            half = S_MAX // 2
            nc.sync.dma_start(out=K_T[:, :half], in_=kT_d[:, :half])
            nc.sync.dma_start(out=v_sb[:, :ST // 2], in_=v_d[:, :ST // 2])
            nc.sync.dma_start(out=K_T[:, half:], in_=kT_d[:, half:])
            nc.sync.dma_start(out=v_sb[:, ST // 2:], in_=v_d[:, ST // 2:])
            nc.sync.dma_start(out=woT_sb[:, :14], in_=woT_d[:, :14])
            nc.sync.dma_start(out=woT_sb[:, 14:], in_=woT_d[:, 14:])

            # ---- RoPE (k first so the cache window is ready early) ----
            def rope(dst, raw, cos_t, sin_t):
                rot_ps = pp.tile([128, L], f32, tag="kv", bufs=2, name="rot_ps")
                nc.tensor.matmul(rot_ps[:], lhsT=rotm, rhs=raw[:],
                                 start=True, stop=True)
                t1 = tmppool.tile([128, L], bf, tag="rt1", name="rt1")
                nc.vector.tensor_mul(t1[:], raw[:], cos_t)
                t2 = tmppool.tile([128, L], bf, tag="rt2", name="rt2")
                nc.vector.tensor_mul(t2[:], rot_ps[:], sin_t)
                nc.vector.tensor_add(dst, t1[:], t2[:])

            # k/v identities first (frees their psum ring slots for the rope
            # rot-MMs), then q0/q1 rope so pair-0 attention starts on the
            # cached s-tiles before the k/v window (needed from st16) lands
            k_raw = tmppool.tile([128, L], bf, tag="kraw", bufs=1, name="k_raw")
            nc.scalar.activation(k_raw[:], k_ps[:], AF.Identity,
                                 bias=bia[:, 4:5])
            v_raw = tmppool.tile([128, L], bf, tag="vraw", bufs=1, name="v_raw")
            nc.scalar.activation(v_raw[:], v_ps[:], AF.Identity,
                                 bias=bia[:, 5:6])

            def qrope(j):
                q_raw = tmppool.tile([128, L], bf, tag="qraw", bufs=2,
                                     name=f"q_raw{j}")
                nc.scalar.activation(q_raw[:], q_ps[j][:], AF.Identity,
                                     bias=bia[:, j:j + 1])
                rope(q_sb[j][:], q_raw, qcos, qsin)

            qrope(0)
            qrope(1)
            rope(K_T[:, cp:cp + L], k_raw, kcos, ksin)
            for lt in range(L // 128):
                tp = pp.tile([128, 128], bf, tag="tp", bufs=2, name=f"tpv{lt}")
                nc.tensor.transpose(tp[:], v_raw[:, lt * 128:(lt + 1) * 128],
                                    ident)
                nc.scalar.copy(v_sb[:, wt0 + lt, :], tp[:])
            qrope(2)
            qrope(3)

        # ---- attention + interleaved o_proj ----
        attg = {}
        scopeB = ExitStack()
        with scopeB:
            pa = scopeB.enter_context(tc.tile_pool(name="pa", bufs=1, space="PSUM"))
            pouts = scopeB.enter_context(tc.tile_pool(name="pouts", bufs=1,
                                                      space="PSUM"))
            ppool = scopeB.enter_context(tc.tile_pool(name="ppool", bufs=3))
            accpool = scopeB.enter_context(tc.tile_pool(name="accpool", bufs=2))
            spool = scopeB.enter_context(tc.tile_pool(name="spool", bufs=2))

            oq = []          # o_proj micro-passes (closures, ~8 PE MMs each)

            def oproj_slot(s):
                cores = [c for c in range(NCORES)
                         if _head_of(c, s) is not None]
                for g in range(4):
                    m0 = g * 128
                    mw = 128 if g < 3 else OSH - 384
                    def pas(s=s, g=g, m0=m0, mw=mw, cores=tuple(cores)):
                        ops = pa.tile([128, L], f32, tag="nop", bufs=2,
                                      name=f"op{s}_{g}")
                        for bi, c in enumerate(cores):
                            gi = REAL_JC.index((s, c))
                            rhs = (attg[0][:, c, s, :] if s < 2
                                   else attg[s][:, c, :])
                            nc.tensor.matmul(
                                ops[0:mw, :],
                                lhsT=woT_sb[:, gi, m0:m0 + mw],
                                rhs=rhs,
                                start=(bi == 0), stop=(bi == len(cores) - 1))
                        if s == 0:
                            nc.vector.tensor_copy(o_acc[0:mw, g, :],
                                                  ops[0:mw, :])
                        else:
                            nc.vector.tensor_add(o_acc[0:mw, g, :],
                                                 o_acc[0:mw, g, :],
                                                 ops[0:mw, :])
                    oq.append(pas)

            def drain_oq(n):
                for _ in range(min(n, len(oq))):
                    oq.pop(0)()

            def norm_slot(key, off, att_un, acc_h):
                nb = pa.tile([128, L], f32, tag="nop", bufs=2,
                             name=f"nb{key}_{off}")
                nc.tensor.matmul(nb[0:1, :], lhsT=ones_f[:], rhs=acc_h,
                                 start=True, stop=True)
                den_sb = spool.tile([1, L], f32, tag="den", name=f"den{key}{off}")
                nc.vector.tensor_copy(den_sb[:], nb[0:1, :])
                rec = spool.tile([1, L], f32, tag="rec", name=f"rec{key}{off}")
                nc.vector.reciprocal_approx_fast(rec[:], den_sb[:])
                nc.tensor.matmul(nb[:, :], lhsT=onesr_f[:], rhs=rec[:],
                                 start=True, stop=True)
                att = spool.tile([128, L], bf, tag="att", name=f"att{key}{off}")
                nc.vector.tensor_mul(att[:], att_un[:], nb[:, :])
                nc.sync.dma_start(out=ag_in[key][off * HD:(off + 1) * HD, :],
                                  in_=att[:])

            def gather(key):
                nc.gpsimd.collective_compute(
                    "AllGather",
                    mybir.AluOpType.bypass,
                    replica_groups=[list(range(NCORES))],
                    ins=[ag_in[key].opt()],
                    outs=[ag_out[key].opt()],
                )

            def read_attg(key, parts):
                nh = 2 if key == 0 else 1
                if nh == 2:
                    agv = ag_out[key].rearrange("(c h p) l -> p c h l",
                                                c=NCORES, h=2, p=128)
                    ag_t = agpool.tile([128, NCORES, 2, L], bf,
                                       tag=f"attg{key}", name=f"attg{key}")
                else:
                    agv = ag_out[key].rearrange("(c p) l -> p c l",
                                                c=NCORES, p=128)
                    ag_t = agpool.tile([128, NCORES, L], bf,
                                       tag=f"attg{key}", name=f"attg{key}")
                step = NCORES // parts
                for q in range(parts):
                    nc.sync.dma_start(out=ag_t[:, q * step:(q + 1) * step],
                                      in_=agv[:, q * step:(q + 1) * step])
                attg[key] = ag_t

            pend = []        # deferred norm/gather/read closures
            for gidx, (a, b) in enumerate(PAIRS):
                outs = [pouts.tile([128, L], f32, tag=f"out{h}",
                                   name=f"out{gidx}_{h}")
                        for h in range(2)]
                acc = accpool.tile([128, 2, L], f32, tag="acc",
                                   name=f"dacc{gidx}")
                p_hist = {}
                for st in range(ST):
                    # deferred norm/gather/read work from the previous pair
                    if st in (1, 3, 5, 7) and pend:
                        pend.pop(0)()
                    sc = pa.tile([128, 2, L], f32, tag="sc", bufs=2,
                                 name=f"sc{gidx}_{st}")
                    kt = K_T[:, st * 128:(st + 1) * 128]
                    nc.tensor.matmul(sc[:, 0, :], lhsT=kt, rhs=q_sb[a][:],
                                     start=True, stop=True)
                    nc.tensor.matmul(sc[:, 1, :], lhsT=kt, rhs=q_sb[b][:],
                                     start=True, stop=True)
                    p = ppool.tile([128, 2, L], bf, tag="p",
                                   name=f"p{gidx}_{st}")
                    nc.scalar.activation(p[:], sc[:], AF.Exp, scale=SCALE)
                    p_hist[st] = p
                    # out-MMs lag one st so exp latency never stalls PE
                    if st > 0:
                        pm, stm = p_hist[st - 1], st - 1
                        vt = v_sb[:, stm, :]
                        nc.tensor.matmul(outs[0][:], lhsT=vt, rhs=pm[:, 0, :],
                                         start=(stm == 0), stop=False)
                        nc.tensor.matmul(outs[1][:], lhsT=vt, rhs=pm[:, 1, :],
                                         start=(stm == 0), stop=False)
                    # softmax denominator: one fold + one accumulate per 2 st
                    if st % 2 == 1:
                        tb = ppool.tile([128, 2, L], bf, tag="tb", bufs=2,
                                        name=f"tb{gidx}_{st}")
                        nc.vector.tensor_add(tb[:], p_hist[st - 1][:], p[:])
                        del p_hist[st - 1]
                        if st == 1:
                            nc.vector.tensor_copy(acc[:], tb[:])
                        else:
                            nc.vector.tensor_add(acc[:], acc[:], tb[:])
                # final out-MMs for st=31
                vt = v_sb[:, ST - 1, :]
                nc.tensor.matmul(outs[0][:], lhsT=vt,
                                 rhs=p_hist[ST - 1][:, 0, :],
                                 start=False, stop=True)
                nc.tensor.matmul(outs[1][:], lhsT=vt,
                                 rhs=p_hist[ST - 1][:, 1, :],
                                 start=False, stop=True)
                # free the out psum banks right away (DVE copies)
                att_un = [spool.tile([128, L], bf, tag=f"attun{h}",
                                     name=f"att_un{gidx}_{h}")
                          for h in range(2)]
                nc.vector.tensor_copy(att_un[0][:], outs[0][:])
                nc.vector.tensor_copy(att_un[1][:], outs[1][:])

                if gidx == 0:
                    pend.append(lambda au=att_un[0], ac=acc:
                                norm_slot(0, 0, au, ac[:, 0, :]))
                    pend.append(lambda au=att_un[1], ac=acc:
                                (norm_slot(0, 1, au, ac[:, 1, :]), gather(0)))
                    pend.append(lambda: read_attg(0, 4))

            # ---- tail: per-slot norms + gathers for slots 2/3; slots 0/1's
            # o_proj (gather-0 data, long since landed) covers the mesh
            # sends of AG2, whose o_proj in turn covers AG3 ----
            norm_slot(2, 0, att_un[0], acc[:, 0, :])
            gather(2)
            norm_slot(3, 0, att_un[1], acc[:, 1, :])
            gather(3)
            oproj_slot(0)
            oproj_slot(1)
            drain_oq(100)
            read_attg(2, 4)
            oproj_slot(2)
            drain_oq(100)
            read_attg(3, 2)
            oproj_slot(3)
            drain_oq(100)

            # output: o_acc -> DRAM per row-tile (host reassembles 448 rows)
            for g in range(4):
                nc.sync.dma_start(out=out_d[:, g * L:(g + 1) * L],
                                  in_=o_acc[:, g, :])

    nc.compile()
    return nc


def _get_prog(cp):
    if cp not in _prog_cache:
        _prog_cache[cp] = _build(cp)
    return _prog_cache[cp]


def _shards(hidden_states, cos, sin, cos_t, sin_t, key_cache, value_cache,
            wq, bq, wk, bk, wv, bv, wo):
    import ml_dtypes
    f = np.float32
    b16 = ml_dtypes.bfloat16

    x = np.asarray(hidden_states, dtype=f).reshape(D, L)
    x3 = np.ascontiguousarray(
        x.reshape(NT, 128, L).transpose(1, 0, 2))          # [128, 28, 512]
    wqT = np.asarray(wq, dtype=f).T                        # [D, H*HD]
    wkT = np.asarray(wk, dtype=f).T                        # [D, KV*HD]
    wvT = np.asarray(wv, dtype=f).T

    qcos = np.asarray(cos_t, dtype=f).reshape(HD, L)
    qsin = np.asarray(sin_t, dtype=f).reshape(HD, L)
    kcos = np.asarray(cos, dtype=f).reshape(L, HD).T
    ksin = np.asarray(sin, dtype=f).reshape(L, HD).T
    trig = np.ascontiguousarray(np.stack([qcos, qsin, kcos, ksin], axis=1))

    rotm = np.zeros((HD, HD), dtype=f)
    half = HD // 2
    rotm[np.arange(half), np.arange(half) + half] = -1.0
    rotm[np.arange(half) + half, np.arange(half)] = 1.0
    idrot = np.ascontiguousarray(
        np.stack([np.eye(HD, dtype=f), rotm.T], axis=1)).astype(b16)

    maps = []
    for c in range(NCORES):
        kvh = c // 2
        proj = np.zeros((128, NT, 1280), dtype=f)
        proj[:, :, 0:512] = x3
        biases = np.zeros((128, 6), dtype=f)
        for s in range(SLOTS):
            h = _head_of(c, s)
            if h is None:
                continue
            wsl = wqT[:, h * HD:(h + 1) * HD]
            proj[:, :, 512 + s * 128:640 + s * 128] = \
                wsl.reshape(NT, 128, HD).transpose(1, 0, 2)
            biases[:, s] = bq[h * HD:(h + 1) * HD]
        proj[:, :, 1024:1152] = wkT[:, kvh * HD:(kvh + 1) * HD] \
            .reshape(NT, 128, HD).transpose(1, 0, 2)
        proj[:, :, 1152:1280] = wvT[:, kvh * HD:(kvh + 1) * HD] \
            .reshape(NT, 128, HD).transpose(1, 0, 2)
        biases[:, 4] = bk[kvh * HD:(kvh + 1) * HD]
        biases[:, 5] = bv[kvh * HD:(kvh + 1) * HD]

        kT = np.ascontiguousarray(np.asarray(key_cache[LI, kvh], dtype=f).T)
        vc = np.ascontiguousarray(
            np.asarray(value_cache[LI, kvh], dtype=f)
            .reshape(ST, 128, HD).transpose(1, 0, 2))

        woT = np.empty((128, len(REAL_JC), OSH), dtype=f)
        rows = slice(OSH * c, OSH * (c + 1))
        for gi, (jj, cc) in enumerate(REAL_JC):
            h = _head_of(cc, jj)
            woT[:, gi, :] = wo[rows, h * HD:(h + 1) * HD].T
        maps.append({
            "proj": proj.astype(b16),
            "trig": trig.astype(b16),
            "biases": np.ascontiguousarray(biases),
            "idrot": idrot,
            "kT": kT.astype(b16),
            "vc": vc.astype(b16),
            "woT": woT.astype(b16),
        })
    return maps


def kernel(_trace=False, **inputs):
    from concourse.bass_utils import run_bass_kernel_spmd

    cp = int(np.asarray(inputs["cache_position"]))
    assert cp % 128 == 0 and 0 <= cp <= S_MAX - L, f"unsupported cache_position {cp}"

    maps = _shards(
        inputs["hidden_states"], inputs["cos"], inputs["sin"],
        inputs["cos_t"], inputs["sin_t"],
        inputs["key_cache"], inputs["value_cache"],
        inputs["wq"], inputs["bq"], inputs["wk"], inputs["bk"],
        inputs["wv"], inputs["bv"], inputs["wo"],
    )
    nc = _get_prog(cp)
    res = run_bass_kernel_spmd(nc, maps, core_ids=list(range(NCORES)),
                               trace=_trace)
    out = np.empty((D, L), dtype=np.float32)
    for c in range(NCORES):
        o = np.asarray(res.results[c]["out"], dtype=np.float32).reshape(128, 4, L)
        for g in range(4):
            mw = 128 if g < 3 else OSH - 384
            out[OSH * c + g * 128: OSH * c + g * 128 + mw] = o[:mw, g]
    out = out.reshape(1, D, 1, L)
    if _trace:
        return out, res
    return out
